# revision 6
# baseline (speedup 1.0000x reference)
"""AudioEncoder Trainium2 kernel.

Computes: conv1d(1->64, k=5, stride=2, pad=2) + bias -> ReLU -> per-timestep
linear (64->64) + bias, over audio [4, 480000] f32 -> out [4, 240000, 64] f32.

Strategy (pure data parallel over 8 cores):
  - Each core handles one half of one batch row: S = 120000 output positions.
  - Host pre-pads/casts audio to fp16 and ships the interleaved padded signal
    xp (xp[t] = x[t-2]); tap k of output position s reads xp[2s + k].
  - On-chip: im2col [5, N] built with one strided DMA per superchunk; conv as
    a K=5 fp16 matmul (two col-group-packed matmuls fill a [128, 512] PSUM
    tile with two 512-position chunks); ACT applies bias+ReLU and evacuates
    PSUM -> SBUF fp16 feats; linear as K=64 fp16 matmuls (feats tile is the
    stationary operand, lin_w.T the moving one) producing [pos, 64] blocks in
    PSUM; DVE adds the (pre-broadcast) linear bias while evacuating to SBUF;
    one contiguous DMA per 1024 positions writes the final [s, p] layout.
"""

import numpy as np

import concourse.bacc as bacc
import concourse.bass as bass
import concourse.mybir as mybir
import concourse.tile as tile
from concourse.bass_utils import run_bass_kernel_spmd

B = 4
T = 480000
S_FULL = 240000  # conv output positions per batch row
N_CORES = 8
S_CORE = S_FULL * B // N_CORES  # 120000 positions per core
CHUNK = 1024  # output positions per inner chunk (two 512 halves)
SUPER = 8192  # output positions covered per im2col DMA
E = 64  # conv out channels
P = 64  # linear out features
KS = 5

f16 = mybir.dt.float16
f32 = mybir.dt.float32


def emit(nc: bass.Bass, S: int = S_CORE) -> None:
    """Emit the per-core Tile kernel for S output positions."""
    from contextlib import ExitStack

    xp_d = nc.declare_dram_parameter("xp", [2 * S + 4], f16, isOutput=False)
    wc_d = nc.declare_dram_parameter("wc", [KS, E], f16, isOutput=False)
    cb_d = nc.declare_dram_parameter("cb", [128, 1], f32, isOutput=False)
    w2_d = nc.declare_dram_parameter("w2", [128, P], f16, isOutput=False)
    b2_d = nc.declare_dram_parameter("b2", [128, 8 * P], f32, isOutput=False)
    out_d = nc.declare_dram_parameter("out", [S, P], f32, isOutput=True)

    RELU = mybir.ActivationFunctionType.Relu
    HALF = CHUNK // 2

    with tile.TileContext(nc) as tc, ExitStack() as ctx:
        consts = ctx.enter_context(tc.tile_pool(name="consts", bufs=1))
        imp = ctx.enter_context(tc.tile_pool(name="im", bufs=2))
        fpool = ctx.enter_context(tc.tile_pool(name="feats", bufs=3))
        opool = ctx.enter_context(tc.tile_pool(name="outs", bufs=3))
        pc = ctx.enter_context(tc.tile_pool(name="psc", bufs=2, space="PSUM"))
        # A and B linear outputs go to separate PSUM banks: row-group-tiled
        # matmuls writing the same partitions of one bank concurrently is a
        # hardware fault (per-partition PSUM write port conflict).
        plA = ctx.enter_context(tc.tile_pool(name="pslA", bufs=2, space="PSUM"))
        plB = ctx.enter_context(tc.tile_pool(name="pslB", bufs=2, space="PSUM"))

        wc_sb = consts.tile([KS, E], f16)
        nc.sync.dma_start(out=wc_sb[:, :], in_=wc_d[:, :])
        cb_sb = consts.tile([128, 1], f32)
        nc.sync.dma_start(out=cb_sb[:, :], in_=cb_d[:, :])
        w2_sb = consts.tile([128, P], f16)
        nc.sync.dma_start(out=w2_sb[:, :], in_=w2_d[:, :])
        b2_sb = consts.tile([128, 8 * P], f32)
        nc.sync.dma_start(out=b2_sb[:, :], in_=b2_d[:, :])

        n_super = (S + SUPER - 1) // SUPER
        for sc in range(n_super):
            sbase = sc * SUPER
            scount = min(SUPER, S - sbase)
            im = imp.tile([KS, SUPER], f16)
            # im[k, j] = xp[2*(sbase + j) + k]; one strided DMA builds all 5
            # overlapping tap rows.
            src = bass.AP(tensor=xp_d, offset=2 * sbase, ap=[[1, KS], [2, scount]])
            nc.sync.dma_start(out=im[:, 0:scount], in_=src)

            cbase = 0
            while cbase < scount:
                cn = min(CHUNK, scount - cbase)
                assert cn % 2 == 0
                nA = cn // 2
                j0 = cbase
                p0g = sbase + cbase  # global first position of this chunk

                # conv: two halves of the chunk land on PSUM partitions
                # 0-63 / 64-127 (col-group packed).
                psc = pc.tile([128, HALF], f32)
                nc.tensor.matmul(
                    out=psc[0:E, 0:nA],
                    lhsT=wc_sb[:, :],
                    rhs=im[:, j0 : j0 + nA],
                    start=True,
                    stop=True,
                )
                nc.tensor.matmul(
                    out=psc[E : 2 * E, 0:nA],
                    lhsT=wc_sb[:, :],
                    rhs=im[:, j0 + nA : j0 + 2 * nA],
                    start=True,
                    stop=True,
                )

                feats = fpool.tile([128, HALF], f16)
                nc.scalar.activation(
                    out=feats[:, 0:nA],
                    in_=psc[:, 0:nA],
                    func=RELU,
                    bias=cb_sb[:, 0:1],
                    scale=1.0,
                )

                # linear: position tiles of <=128 become stationary operands.
                m_tiles = [
                    (i * 128, min(128, nA - i * 128)) for i in range((nA + 127) // 128)
                ]
                mlen0 = m_tiles[0][1]
                assert all(ml == mlen0 for _, ml in m_tiles)
                nb = len(m_tiles)
                psA = plA.tile([128, HALF // 2], f32)
                psB = plB.tile([128, HALF // 2], f32)
                for bi, (mo, ml) in enumerate(m_tiles):
                    nc.tensor.matmul(
                        out=psA[0:ml, P * bi : P * bi + P],
                        lhsT=feats[0:E, mo : mo + ml],
                        rhs=w2_sb[0:E, :],
                        start=True,
                        stop=True,
                    )
                    nc.tensor.matmul(
                        out=psB[0:ml, P * bi : P * bi + P],
                        lhsT=feats[E : 2 * E, mo : mo + ml],
                        rhs=w2_sb[E : 2 * E, :],
                        start=True,
                        stop=True,
                    )

                ncols = nb * P
                outt = opool.tile([128, HALF], f32)
                nc.vector.tensor_add(
                    outt[0:mlen0, 0:ncols],
                    psA[0:mlen0, 0:ncols],
                    b2_sb[0:mlen0, 0:ncols],
                )
                nc.vector.tensor_add(
                    outt[0:mlen0, ncols : 2 * ncols],
                    psB[0:mlen0, 0:ncols],
                    b2_sb[0:mlen0, 0:ncols],
                )

                dview = out_d[p0g : p0g + cn, :].rearrange(
                    "(t r) p -> r t p", r=mlen0
                )
                sview = outt[0:mlen0, 0 : 2 * ncols].rearrange(
                    "r (t p) -> r t p", p=P
                )
                nc.sync.dma_start(out=dview, in_=sview)

                cbase += cn


def prep_inputs(audio_waveform, conv_w, conv_b, lin_w, lin_b):
    """Host-side shard + dtype/layout prep. Returns in_maps for the 8 cores."""
    x = np.asarray(audio_waveform, dtype=np.float32)
    assert x.shape == (B, T)
    xp = np.zeros((B, 2 * S_FULL + 4), dtype=np.float16)
    xp[:, 2 : 2 + T] = x.astype(np.float16)

    conv_w = np.asarray(conv_w, dtype=np.float32)
    conv_b = np.asarray(conv_b, dtype=np.float32)
    lin_w = np.asarray(lin_w, dtype=np.float32)
    lin_b = np.asarray(lin_b, dtype=np.float32)

    wc = np.ascontiguousarray(conv_w[:, 0, :].T).astype(np.float16)  # [5, 64]
    cb = np.ascontiguousarray(
        np.concatenate([conv_b, conv_b]).astype(np.float32)[:, None]
    )  # [128, 1]
    w2 = lin_w.T.astype(np.float16)  # [64e, 64p]
    w2s = np.ascontiguousarray(np.concatenate([w2, w2], axis=0))  # [128, 64]
    b2 = np.ascontiguousarray(
        np.tile(lin_b.astype(np.float32)[None, :], (128, 8))
    )  # [128, 512]

    in_maps = []
    for c in range(N_CORES):
        b_i, h = divmod(c, 2)
        s0 = h * S_CORE
        xpc = np.ascontiguousarray(xp[b_i, 2 * s0 : 2 * s0 + 2 * S_CORE + 4])
        in_maps.append(dict(xp=xpc, wc=wc, cb=cb, w2=w2s, b2=b2))
    return in_maps


_NC_CACHE = None


def get_nc() -> bass.Bass:
    global _NC_CACHE
    if _NC_CACHE is None:
        nc = bacc.Bacc()
        emit(nc)
        # Legalizes TRN2 sync constraints (splits multi-wait instructions),
        # allocates registers, etc. Required before walrus codegen.
        nc.compile()
        _NC_CACHE = nc
    return _NC_CACHE


def run(inputs: dict, trace: bool = False):
    """Run on the 8 cores; returns (full_output, BassKernelResults)."""
    in_maps = prep_inputs(**inputs)
    nc = get_nc()
    res = run_bass_kernel_spmd(nc, in_maps, list(range(N_CORES)), trace=trace)
    out = np.empty((B, S_FULL, P), dtype=np.float32)
    for c in range(N_CORES):
        b_i, h = divmod(c, 2)
        out[b_i, h * S_CORE : (h + 1) * S_CORE, :] = res.results[c]["out"]
    return out, res


def kernel(**inputs) -> np.ndarray:
    out, _ = run(inputs)
    return out
